# revision 2
# baseline (speedup 1.0000x reference)
"""ConcatCritic fused pair-grid MLP on 8 Trainium2 NeuronCores — v2 (fp16).

Math (reference):
    hx = x @ W1[:DX]                      # [B, H]
    hy = y @ W1[DX:] + b1                 # [B, H]
    h  = relu(hx[:,None,:] + hy[None,:,:])        # [B, B, H]
    h2 = relu(h @ W2 + b2)                        # [B, B, H]
    out[i, j] = (h2 @ W3)[i, j, 0] + b3           # [B, B]

Sharding: data-parallel over i (x rows). Each of the 8 cores computes a
[64, 512] slab of scores. y / W1 / W2 / W3 / b* are replicated.

v2 changes vs v1:
  * All matmul operands and SBUF activations are fp16 (PSUM stays f32).
    Halves SBUF read/write bandwidth on every engine and avoids fp32r's
    hardware throughput cliff. Error budget: fp16 rounding ~5e-4 per
    tensor, fp32 accumulation -> ~1e-3 total vs the 2e-2 gate.
  * Input pack is fp16 and loaded by TWO parallel HWDGE DMAs (SP +
    Activation queues) instead of one SWDGE DMA; the small f32 bias pack
    rides the SP queue first. Prep operands land on one queue, W2/W3 on
    the other, so prep GEMMs start before the whole pack arrives.

Per-core dataflow:
    prep:  hyT[h, j]  = (W1y.T @ yT)           [256, 512] fp16 in SBUF
           bias[h, i] = (W1x.T @ xT) + b1      [256, 64]  f32  in SBUF
    per i: A_iT[h, j]  = relu(hyT + bias[:, i])     (DVE, fp16 out)
           Z_iT[m, j]  = W2.T @ A_iT   (PSUM f32)   (4 matmuls, N=512)
           Z2_iT[m, j] = relu(Z_iT + b2)            (ACT, fp16 out)
           score[i, j] += W3.T @ Z2_iT              (2 matmuls)
    The 128 score matvecs accumulate into ONE [64, 512] PSUM tile via the
    shifted one-hot W3 trick (matvec i writes only partition i). b3 is
    added on the host after the gather.

Sync-wait discipline: walrus permits only ONE semaphore wait per compute
instruction. Each engine "pre-touches" every DMA'd region once so later
instructions never need a fresh DMA wait; excess waits are spilled into
EventSemaphore instructions by _legalize_waits.
"""

import numpy as np

B = 512
DX = 128
DY = 128
H = 256
P = 128          # partitions
HC = H // P      # h chunks (2)
NCORES = 8
BS = B // NCORES  # 64 rows of x per core

# fp16 pack column offsets (fp16 words per partition)
# queue A (SP):  prep operands  [OFF_YT .. OFF_W1Y+H)
# queue B (ACT): W2 + W3 trick  [OFF_W2 .. OFF_W3T+2P)
OFF_YT = 0            # [512]     y.T
OFF_XT = 512          # [64]      x_shard.T
OFF_W1X = 576         # [256]     W1[:DX]
OFF_W1Y = 832         # [256]     W1[DX:]
SPLIT_A = 1088        # end of queue-A region
OFF_W2 = 1088         # [2, 256]  W2 k-chunks      (lhsT slices [128,128])
OFF_W3T = 1600        # [2, 128]  one-hot W3 trick
PACK16_COLS = 1856

PACK32_COLS = 4       # b1 (2 cols), b2 (2 cols)

_cache = {}


def _build_nc(legalize=True, reps=1, loop_reps=0):
    import concourse.bass as bass
    import concourse.tile as tile
    import concourse.mybir as mybir

    f32 = mybir.dt.float32
    f16 = mybir.dt.float16
    Alu = mybir.AluOpType
    Act = mybir.ActivationFunctionType

    nc = bass.Bass(
        trn_type="TRN2",
        target_bir_lowering=False,
        debug=False,
        num_devices=NCORES,
    )

    d_pack16 = nc.dram_tensor("pack16", [P, PACK16_COLS], f16, kind="ExternalInput")
    d_pack32 = nc.dram_tensor("pack32", [P, PACK32_COLS], f32, kind="ExternalInput")
    d_out = nc.dram_tensor("out", [BS, B], f32, kind="ExternalOutput")

    with tile.TileContext(nc) as tc:
        with (
            tc.tile_pool(name="singles", bufs=1) as singles,
            tc.tile_pool(name="apool0", bufs=8) as apool0,
            tc.tile_pool(name="apool1", bufs=8) as apool1,
            tc.tile_pool(name="z2pool0", bufs=6) as z2pool0,
            tc.tile_pool(name="z2pool1", bufs=6) as z2pool1,
            tc.tile_pool(name="zpool", bufs=3, space="PSUM") as zpool,
            tc.tile_pool(name="spool", bufs=1, space="PSUM") as spool,
        ):
            pk = singles.tile([P, PACK16_COLS], f16)
            pk32 = singles.tile([P, PACK32_COLS], f32)
            nc.sync.dma_start(pk32[:], d_pack32[:])
            nc.sync.dma_start(pk[:, :SPLIT_A], d_pack16[:, :SPLIT_A])
            nc.scalar.dma_start(pk[:, SPLIT_A:], d_pack16[:, SPLIT_A:])

            def w2_lhsT(c, m):
                return pk[:, OFF_W2 + c * H + m * P: OFF_W2 + c * H + (m + 1) * P]

            def w3_lhsT(c, i):
                o = OFF_W3T + c * P + P // 2 - i
                return pk[:, o: o + BS]

            def b1_col(c):
                return pk32[:, c: c + 1]

            def b2_col(c):
                return pk32[:, HC + c: HC + c + 1]

            # pre-touch: one op per engine per DMA'd region it reads, so
            # later instructions never need a fresh DMA wait.
            scratch = singles.tile([P, 4], f32)
            nc.vector.tensor_copy(scratch[:, 0:1], b1_col(0))
            nc.scalar.copy(scratch[:, 1:2], b2_col(0))
            nc.vector.tensor_copy(scratch[:, 2:3].bitcast(f16), pk[:, 0:2])
            nc.scalar.copy(scratch[:, 3:4].bitcast(f16), pk[:, SPLIT_A:SPLIT_A + 2])

            sb_hy = singles.tile([P, HC, B], f16)
            sb_bias = singles.tile([P, HC, BS], f32)
            outbuf = singles.tile([BS, B], f32)

            # ---- prep: hyT and per-row bias ----
            for c in range(HC):
                ps_hy = zpool.tile([P, B], f32, tag="z")
                nc.tensor.matmul(
                    ps_hy[:],
                    pk[:, OFF_W1Y + c * P: OFF_W1Y + (c + 1) * P],
                    pk[:, OFF_YT: OFF_YT + B],
                    start=True,
                    stop=True,
                )
                nc.vector.tensor_copy(sb_hy[:, c, :], ps_hy[:])

                ps_hx = zpool.tile([P, BS], f32, tag="z")
                nc.tensor.matmul(
                    ps_hx[:],
                    pk[:, OFF_W1X + c * P: OFF_W1X + (c + 1) * P],
                    pk[:, OFF_XT: OFF_XT + BS],
                    start=True,
                    stop=True,
                )
                nc.vector.tensor_scalar(
                    sb_bias[:, c, :], ps_hx[:], b1_col(c), None, Alu.add
                )

            score_ps = spool.tile([BS, B], f32)

            # ---- main loop over the 64 x-rows of this core ----
            # (reps>1 replicates the whole loop for slope-based device
            # timing; each rep restarts the score accumulation;
            # loop_reps>0 wraps the body in a device-side For_i instead)
            import contextlib
            loop_cm = (
                tc.For_i(0, loop_reps, 1) if loop_reps
                else contextlib.nullcontext()
            )
            # Software pipeline: row idx's W3 score matvecs are emitted
            # after row idx+1's W2 matmuls, so the PE never waits on the
            # ACT relu of the row it is scoring (z2 is ~a full W2 stage
            # old by then). `pending` holds (i, z20, z21) of the row whose
            # W3 stage hasn't been emitted yet.
            with loop_cm:
              for rep in range(reps):
                pending = None
                n_sc = 0  # score matmuls emitted (for start flag)
                for idx in range(BS):
                  i = (idx % 2) * (BS // 2) + idx // 2
                  a0 = apool0.tile([P, B], f16, tag="a0")
                  a1 = apool1.tile([P, B], f16, tag="a1")
                  a = [a0, a1]
                  nc.vector.tensor_scalar(
                      a1[:], sb_hy[:, 1, :], sb_bias[:, 1, i:i + 1],
                      0.0, Alu.add, Alu.max,
                  )
                  nc.vector.tensor_scalar(
                      a0[:], sb_hy[:, 0, :], sb_bias[:, 0, i:i + 1],
                      0.0, Alu.add, Alu.max,
                  )

                  z = zpool.tile([P, HC, B], f32, tag="z")
                  for m in range(HC):
                      for c in range(HC):
                          nc.tensor.matmul(
                              z[:, m, :],
                              w2_lhsT(c, m),
                              a[c][:],
                              start=(c == 0),
                              stop=(c == HC - 1),
                              skip_group_check=True,
                          )

                  if pending is not None:
                      pi, pz2 = pending
                      for c in range(HC):
                          nc.tensor.matmul(
                              score_ps[:],
                              w3_lhsT(c, pi),
                              pz2[c][:],
                              start=(n_sc == 0),
                              stop=False,
                              skip_group_check=True,
                          )
                          n_sc += 1

                  z20 = z2pool0.tile([P, B], f16, tag="z20")
                  z21 = z2pool1.tile([P, B], f16, tag="z21")
                  nc.scalar.activation(
                      z21[:], z[:, 1, :], Act.Relu, bias=b2_col(1), scale=1.0
                  )
                  nc.scalar.activation(
                      z20[:], z[:, 0, :], Act.Relu, bias=b2_col(0), scale=1.0
                  )
                  pending = (i, [z20, z21])

                # drain: W3 for the last row
                pi, pz2 = pending
                for c in range(HC):
                    nc.tensor.matmul(
                        score_ps[:],
                        w3_lhsT(c, pi),
                        pz2[c][:],
                        start=(n_sc == 0),
                        stop=(c == HC - 1),
                        skip_group_check=True,
                    )
                    n_sc += 1

            nc.vector.tensor_copy(outbuf[:], score_ps[:])
            nc.sync.dma_start(d_out[:], outbuf[:])

    if legalize:
        _legalize_waits(nc)
    return nc


def _legalize_waits(nc):
    """walrus accepts only ONE sync wait per compute instruction (and two
    per EventSemaphore). First DROP same-engine self-waits (a wait on the
    engine's own completion semaphore with threshold <= the number of
    preceding instructions on that queue is trivially satisfied by queue
    order); then spill any remaining excess into EventSemaphore
    instructions inserted just before the op on the same engine queue."""
    import concourse.mybir as mybir

    # For each semaphore, track (a) how much it has been incremented by
    # PRECEDING instructions on each engine queue, and (b) whether any
    # other engine also updates it. A wait on a sem that is updated only
    # by the waiting instruction's own queue, with threshold <= the
    # increments already emitted before it, is trivially satisfied by
    # in-order queue execution and can be dropped.
    sem_engines = {}
    for f in nc.m.functions:
        for bb in f.blocks:
            for inst in bb.instructions:
                si = inst.sync_info
                if si is None:
                    continue
                for u in si.on_update or []:
                    sem_engines.setdefault(u.ant_name, set()).add(str(inst.engine))

    n_spilled = 0
    for f in nc.m.functions:
        for bb in f.blocks:
            upd = {}
            insts = bb.instructions
            i = 0
            while i < len(insts):
                inst = insts[i]
                eng = str(inst.engine)
                si = inst.sync_info
                if si is None:
                    i += 1
                    continue
                waits = list(si.on_wait or [])
                if len(waits) > 1:
                    kept = [
                        w for w in waits
                        if not (
                            w.wait_mode == "sem-ge-imm"
                            and sem_engines.get(w.ant_name) == {eng}
                            and (w.wait_value or 0) <= upd.get(w.ant_name, 0)
                        )
                    ]
                    if kept:
                        waits = kept
                if len(waits) != len(si.on_wait or []):
                    inst.sync_info = mybir.SyncInfo(
                        on_wait=waits, on_update=list(si.on_update or [])
                    )
                    si = inst.sync_info
                for u in si.on_update or []:
                    if u.update_mode == "sem-inc":
                        upd[u.ant_name] = upd.get(u.ant_name, 0) + (
                            u.update_value or 0
                        )
                if not si.on_wait or len(si.on_wait) <= 1 or (
                    inst.opcode == "EventSemaphore"
                ):
                    i += 1
                    continue
                keep, spill = waits[-1], waits[:-1]
                k = 0
                while spill:
                    chunk, spill = spill[:2], spill[2:]
                    ev = mybir.InstEventSemaphore(
                        name=f"{inst.name}-lw{k}", ins=[], outs=[]
                    )
                    ev.engine = inst.engine
                    ev.sync_info = mybir.SyncInfo(on_wait=chunk, on_update=[])
                    insts.insert(i, ev)
                    i += 1
                    k += 1
                    n_spilled += 1
                inst.sync_info = mybir.SyncInfo(
                    on_wait=[keep], on_update=list(si.on_update or [])
                )
                i += 1
    return n_spilled


def prep_inputs(x, y, W1, b1, W2, b2, W3):
    """Host-side sharding/layout. Returns per-core input maps."""
    x = np.ascontiguousarray(np.asarray(x, dtype=np.float32))
    y = np.ascontiguousarray(np.asarray(y, dtype=np.float32))
    W1 = np.asarray(W1, dtype=np.float32)
    b1 = np.asarray(b1, dtype=np.float32)
    W2 = np.asarray(W2, dtype=np.float32)
    b2 = np.asarray(b2, dtype=np.float32)
    W3 = np.asarray(W3, dtype=np.float32)

    pack = np.zeros((P, PACK16_COLS), dtype=np.float16)
    pack[:, OFF_YT:OFF_YT + B] = y.T
    pack[:, OFF_W1X:OFF_W1X + H] = W1[:DX]
    pack[:, OFF_W1Y:OFF_W1Y + H] = W1[DX:]
    # W2 k-chunks: pack[p, OFF_W2 + c*H + j] = W2[c*P + p, j]
    pack[:, OFF_W2:OFF_W2 + HC * H] = (
        np.transpose(W2.reshape(HC, P, H), (1, 0, 2)).reshape(P, HC * H)
    )
    # one-hot W3 trick: column 64 of each [128,128] region holds W3 chunk c
    for c in range(HC):
        pack[:, OFF_W3T + c * P + P // 2] = W3[c * P:(c + 1) * P, 0]

    pack32 = np.zeros((P, PACK32_COLS), dtype=np.float32)
    pack32[:, 0:HC] = b1.reshape(HC, P).T
    pack32[:, HC:2 * HC] = b2.reshape(HC, P).T

    in_maps = []
    for core in range(NCORES):
        pc = pack.copy()
        pc[:, OFF_XT:OFF_XT + BS] = x[core * BS:(core + 1) * BS].T.astype(np.float16)
        in_maps.append({"pack16": pc, "pack32": pack32})
    return in_maps


def kernel(x, y, W1, b1, W2, b2, W3, b3):
    from concourse.bass_utils import run_bass_kernel_spmd

    if "nc" not in _cache:
        _cache["nc"] = _build_nc()
    nc = _cache["nc"]

    in_maps = prep_inputs(x, y, W1, b1, W2, b2, W3)
    res = run_bass_kernel_spmd(nc, in_maps, core_ids=list(range(NCORES)))
    out = np.concatenate([res.results[c]["out"] for c in range(NCORES)], axis=0)
    out = out + np.float32(np.asarray(b3, dtype=np.float32).reshape(()))
    return out.astype(np.float32)


# revision 3
# speedup vs baseline: 1.0235x; 1.0235x over previous
"""ConcatCritic fused pair-grid MLP on 8 Trainium2 NeuronCores — v2 (fp16).

Math (reference):
    hx = x @ W1[:DX]                      # [B, H]
    hy = y @ W1[DX:] + b1                 # [B, H]
    h  = relu(hx[:,None,:] + hy[None,:,:])        # [B, B, H]
    h2 = relu(h @ W2 + b2)                        # [B, B, H]
    out[i, j] = (h2 @ W3)[i, j, 0] + b3           # [B, B]

Sharding: data-parallel over i (x rows). Each of the 8 cores computes a
[64, 512] slab of scores. y / W1 / W2 / W3 / b* are replicated.

v2 changes vs v1:
  * All matmul operands and SBUF activations are fp16 (PSUM stays f32).
    Halves SBUF read/write bandwidth on every engine and avoids fp32r's
    hardware throughput cliff. Error budget: fp16 rounding ~5e-4 per
    tensor, fp32 accumulation -> ~1e-3 total vs the 2e-2 gate.
  * Input pack is fp16 and loaded by TWO parallel HWDGE DMAs (SP +
    Activation queues) instead of one SWDGE DMA; the small f32 bias pack
    rides the SP queue first. Prep operands land on one queue, W2/W3 on
    the other, so prep GEMMs start before the whole pack arrives.

Per-core dataflow:
    prep:  hyT[h, j]  = (W1y.T @ yT)           [256, 512] fp16 in SBUF
           bias[h, i] = (W1x.T @ xT) + b1      [256, 64]  f32  in SBUF
    per i: A_iT[h, j]  = relu(hyT + bias[:, i])     (DVE, fp16 out)
           Z_iT[m, j]  = W2.T @ A_iT   (PSUM f32)   (4 matmuls, N=512)
           Z2_iT[m, j] = relu(Z_iT + b2)            (ACT, fp16 out)
           score[i, j] += W3.T @ Z2_iT              (2 matmuls)
    The 128 score matvecs accumulate into ONE [64, 512] PSUM tile via the
    shifted one-hot W3 trick (matvec i writes only partition i). b3 is
    added on the host after the gather.

Sync-wait discipline: walrus permits only ONE semaphore wait per compute
instruction. Each engine "pre-touches" every DMA'd region once so later
instructions never need a fresh DMA wait; excess waits are spilled into
EventSemaphore instructions by _legalize_waits.
"""

import numpy as np

B = 512
DX = 128
DY = 128
H = 256
P = 128          # partitions
HC = H // P      # h chunks (2)
NCORES = 8
BS = B // NCORES  # 64 rows of x per core

# fp16 pack column offsets (fp16 words per partition)
# queue A (SP):  prep operands  [OFF_YT .. OFF_W1Y+H)
# queue B (ACT): W2 + W3 trick  [OFF_W2 .. OFF_W3T+2P)
OFF_YT = 0            # [512]     y.T
OFF_XT = 512          # [64]      x_shard.T
OFF_W1X = 576         # [256]     W1[:DX]
OFF_W1Y = 832         # [256]     W1[DX:]
SPLIT_A = 1088        # end of queue-A region
OFF_W2 = 1088         # [2, 256]  W2 k-chunks      (lhsT slices [128,128])
OFF_W3T = 1600        # [2, 128]  one-hot W3 trick
PACK16_COLS = 1856

PACK32_COLS = 4       # b1 (2 cols), b2 (2 cols)

_cache = {}


def _build_nc(legalize=True, reps=1, loop_reps=0):
    import concourse.bass as bass
    import concourse.tile as tile
    import concourse.mybir as mybir

    f32 = mybir.dt.float32
    f16 = mybir.dt.float16
    Alu = mybir.AluOpType
    Act = mybir.ActivationFunctionType

    nc = bass.Bass(
        trn_type="TRN2",
        target_bir_lowering=False,
        debug=False,
        num_devices=NCORES,
    )

    d_pack16 = nc.dram_tensor("pack16", [P, PACK16_COLS], f16, kind="ExternalInput")
    d_pack32 = nc.dram_tensor("pack32", [P, PACK32_COLS], f32, kind="ExternalInput")
    d_out = nc.dram_tensor("out", [BS, B], f32, kind="ExternalOutput")

    with tile.TileContext(nc) as tc:
        with (
            tc.tile_pool(name="singles", bufs=1) as singles,
            tc.tile_pool(name="apool0", bufs=8) as apool0,
            tc.tile_pool(name="apool1", bufs=8) as apool1,
            tc.tile_pool(name="z2pool0", bufs=6) as z2pool0,
            tc.tile_pool(name="z2pool1", bufs=6) as z2pool1,
            tc.tile_pool(name="zpool", bufs=3, space="PSUM") as zpool,
            tc.tile_pool(name="spool", bufs=1, space="PSUM") as spool,
        ):
            pk = singles.tile([P, PACK16_COLS], f16)
            pk32 = singles.tile([P, PACK32_COLS], f32)
            nc.sync.dma_start(pk32[:], d_pack32[:])
            nc.sync.dma_start(pk[:, :SPLIT_A], d_pack16[:, :SPLIT_A])
            nc.scalar.dma_start(pk[:, SPLIT_A:], d_pack16[:, SPLIT_A:])

            def w2_lhsT(c, m):
                return pk[:, OFF_W2 + c * H + m * P: OFF_W2 + c * H + (m + 1) * P]

            def w3_lhsT(c, i):
                o = OFF_W3T + c * P + P // 2 - i
                return pk[:, o: o + BS]

            def b1_col(c):
                return pk32[:, c: c + 1]

            def b2_col(c):
                return pk32[:, HC + c: HC + c + 1]

            # pre-touch: one op per engine per DMA'd region it reads, so
            # later instructions never need a fresh DMA wait.
            scratch = singles.tile([P, 4], f32)
            nc.vector.tensor_copy(scratch[:, 0:1], b1_col(0))
            nc.scalar.copy(scratch[:, 1:2], b2_col(0))
            nc.vector.tensor_copy(scratch[:, 2:3].bitcast(f16), pk[:, 0:2])
            nc.scalar.copy(scratch[:, 3:4].bitcast(f16), pk[:, SPLIT_A:SPLIT_A + 2])

            sb_hy = singles.tile([P, HC, B], f16)
            sb_bias = singles.tile([P, HC, BS], f32)
            outbuf = singles.tile([BS, B], f32)

            # ---- prep: hyT and per-row bias ----
            for c in range(HC):
                ps_hy = zpool.tile([P, B], f32, tag="z")
                nc.tensor.matmul(
                    ps_hy[:],
                    pk[:, OFF_W1Y + c * P: OFF_W1Y + (c + 1) * P],
                    pk[:, OFF_YT: OFF_YT + B],
                    start=True,
                    stop=True,
                )
                nc.vector.tensor_copy(sb_hy[:, c, :], ps_hy[:])

                ps_hx = zpool.tile([P, BS], f32, tag="z")
                nc.tensor.matmul(
                    ps_hx[:],
                    pk[:, OFF_W1X + c * P: OFF_W1X + (c + 1) * P],
                    pk[:, OFF_XT: OFF_XT + BS],
                    start=True,
                    stop=True,
                )
                nc.vector.tensor_scalar(
                    sb_bias[:, c, :], ps_hx[:], b1_col(c), None, Alu.add
                )

            score_ps = spool.tile([BS, B], f32)

            # ---- main loop over the 64 x-rows of this core ----
            # (reps>1 replicates the whole loop for slope-based device
            # timing; each rep restarts the score accumulation;
            # loop_reps>0 wraps the body in a device-side For_i instead)
            import contextlib
            loop_cm = (
                tc.For_i(0, loop_reps, 1) if loop_reps
                else contextlib.nullcontext()
            )
            # Software pipeline: row idx's W3 score matvecs are emitted
            # after row idx+1's W2 matmuls, so the PE never waits on the
            # ACT relu of the row it is scoring (z2 is ~a full W2 stage
            # old by then). `pending` holds (i, z20, z21) of the row whose
            # W3 stage hasn't been emitted yet.
            with loop_cm:
              for rep in range(reps):
                pend_q = []
                n_sc = 0  # score matmuls emitted (for start flag)
                for idx in range(BS):
                  i = (idx % 2) * (BS // 2) + idx // 2
                  a0 = apool0.tile([P, B], f16, tag="a0")
                  a1 = apool1.tile([P, B], f16, tag="a1")
                  a = [a0, a1]
                  nc.vector.tensor_scalar(
                      a1[:], sb_hy[:, 1, :], sb_bias[:, 1, i:i + 1],
                      0.0, Alu.add, Alu.max,
                  )
                  nc.vector.tensor_scalar(
                      a0[:], sb_hy[:, 0, :], sb_bias[:, 0, i:i + 1],
                      0.0, Alu.add, Alu.max,
                  )

                  z = zpool.tile([P, HC, B], f32, tag="z")
                  for m in range(HC):
                      for c in range(HC):
                          nc.tensor.matmul(
                              z[:, m, :],
                              w2_lhsT(c, m),
                              a[c][:],
                              start=(c == 0),
                              stop=(c == HC - 1),
                              skip_group_check=True,
                          )

                  if len(pend_q) >= 2:
                      pi, pz2 = pend_q.pop(0)
                      for c in range(HC):
                          nc.tensor.matmul(
                              score_ps[:],
                              w3_lhsT(c, pi),
                              pz2[c][:],
                              start=(n_sc == 0),
                              stop=False,
                              skip_group_check=True,
                          )
                          n_sc += 1

                  z20 = z2pool0.tile([P, B], f16, tag="z20")
                  z21 = z2pool1.tile([P, B], f16, tag="z21")
                  nc.scalar.activation(
                      z21[:], z[:, 1, :], Act.Relu, bias=b2_col(1), scale=1.0
                  )
                  nc.scalar.activation(
                      z20[:], z[:, 0, :], Act.Relu, bias=b2_col(0), scale=1.0
                  )
                  pend_q.append((i, [z20, z21]))

                # drain: W3 for the remaining rows
                n_drain = len(pend_q)
                for k in range(n_drain):
                    pi, pz2 = pend_q.pop(0)
                    for c in range(HC):
                        nc.tensor.matmul(
                            score_ps[:],
                            w3_lhsT(c, pi),
                            pz2[c][:],
                            start=(n_sc == 0),
                            stop=(k == n_drain - 1 and c == HC - 1),
                            skip_group_check=True,
                        )
                        n_sc += 1

            nc.vector.tensor_copy(outbuf[:], score_ps[:])
            nc.sync.dma_start(d_out[:], outbuf[:])

    if legalize:
        _legalize_waits(nc)
    return nc


def _legalize_waits(nc):
    """walrus accepts only ONE sync wait per compute instruction (and two
    per EventSemaphore). First DROP same-engine self-waits (a wait on the
    engine's own completion semaphore with threshold <= the number of
    preceding instructions on that queue is trivially satisfied by queue
    order); then spill any remaining excess into EventSemaphore
    instructions inserted just before the op on the same engine queue."""
    import concourse.mybir as mybir

    # For each semaphore, track (a) how much it has been incremented by
    # PRECEDING instructions on each engine queue, and (b) whether any
    # other engine also updates it. A wait on a sem that is updated only
    # by the waiting instruction's own queue, with threshold <= the
    # increments already emitted before it, is trivially satisfied by
    # in-order queue execution and can be dropped.
    sem_engines = {}
    for f in nc.m.functions:
        for bb in f.blocks:
            for inst in bb.instructions:
                si = inst.sync_info
                if si is None:
                    continue
                for u in si.on_update or []:
                    sem_engines.setdefault(u.ant_name, set()).add(str(inst.engine))

    n_spilled = 0
    for f in nc.m.functions:
        for bb in f.blocks:
            upd = {}
            insts = bb.instructions
            i = 0
            while i < len(insts):
                inst = insts[i]
                eng = str(inst.engine)
                si = inst.sync_info
                if si is None:
                    i += 1
                    continue
                waits = list(si.on_wait or [])
                if len(waits) > 1:
                    kept = [
                        w for w in waits
                        if not (
                            w.wait_mode == "sem-ge-imm"
                            and sem_engines.get(w.ant_name) == {eng}
                            and (w.wait_value or 0) <= upd.get(w.ant_name, 0)
                        )
                    ]
                    if kept:
                        waits = kept
                if len(waits) != len(si.on_wait or []):
                    inst.sync_info = mybir.SyncInfo(
                        on_wait=waits, on_update=list(si.on_update or [])
                    )
                    si = inst.sync_info
                for u in si.on_update or []:
                    if u.update_mode == "sem-inc":
                        upd[u.ant_name] = upd.get(u.ant_name, 0) + (
                            u.update_value or 0
                        )
                if not si.on_wait or len(si.on_wait) <= 1 or (
                    inst.opcode == "EventSemaphore"
                ):
                    i += 1
                    continue
                keep, spill = waits[-1], waits[:-1]
                k = 0
                while spill:
                    chunk, spill = spill[:2], spill[2:]
                    ev = mybir.InstEventSemaphore(
                        name=f"{inst.name}-lw{k}", ins=[], outs=[]
                    )
                    ev.engine = inst.engine
                    ev.sync_info = mybir.SyncInfo(on_wait=chunk, on_update=[])
                    insts.insert(i, ev)
                    i += 1
                    k += 1
                    n_spilled += 1
                inst.sync_info = mybir.SyncInfo(
                    on_wait=[keep], on_update=list(si.on_update or [])
                )
                i += 1
    return n_spilled


def prep_inputs(x, y, W1, b1, W2, b2, W3):
    """Host-side sharding/layout. Returns per-core input maps."""
    x = np.ascontiguousarray(np.asarray(x, dtype=np.float32))
    y = np.ascontiguousarray(np.asarray(y, dtype=np.float32))
    W1 = np.asarray(W1, dtype=np.float32)
    b1 = np.asarray(b1, dtype=np.float32)
    W2 = np.asarray(W2, dtype=np.float32)
    b2 = np.asarray(b2, dtype=np.float32)
    W3 = np.asarray(W3, dtype=np.float32)

    pack = np.zeros((P, PACK16_COLS), dtype=np.float16)
    pack[:, OFF_YT:OFF_YT + B] = y.T
    pack[:, OFF_W1X:OFF_W1X + H] = W1[:DX]
    pack[:, OFF_W1Y:OFF_W1Y + H] = W1[DX:]
    # W2 k-chunks: pack[p, OFF_W2 + c*H + j] = W2[c*P + p, j]
    pack[:, OFF_W2:OFF_W2 + HC * H] = (
        np.transpose(W2.reshape(HC, P, H), (1, 0, 2)).reshape(P, HC * H)
    )
    # one-hot W3 trick: column 64 of each [128,128] region holds W3 chunk c
    for c in range(HC):
        pack[:, OFF_W3T + c * P + P // 2] = W3[c * P:(c + 1) * P, 0]

    pack32 = np.zeros((P, PACK32_COLS), dtype=np.float32)
    pack32[:, 0:HC] = b1.reshape(HC, P).T
    pack32[:, HC:2 * HC] = b2.reshape(HC, P).T

    in_maps = []
    for core in range(NCORES):
        pc = pack.copy()
        pc[:, OFF_XT:OFF_XT + BS] = x[core * BS:(core + 1) * BS].T.astype(np.float16)
        in_maps.append({"pack16": pc, "pack32": pack32})
    return in_maps


def kernel(x, y, W1, b1, W2, b2, W3, b3):
    from concourse.bass_utils import run_bass_kernel_spmd

    if "nc" not in _cache:
        _cache["nc"] = _build_nc()
    nc = _cache["nc"]

    in_maps = prep_inputs(x, y, W1, b1, W2, b2, W3)
    res = run_bass_kernel_spmd(nc, in_maps, core_ids=list(range(NCORES)))
    out = np.concatenate([res.results[c]["out"] for c in range(NCORES)], axis=0)
    out = out + np.float32(np.asarray(b3, dtype=np.float32).reshape(()))
    return out.astype(np.float32)


# revision 4
# speedup vs baseline: 1.2643x; 1.2353x over previous
"""ConcatCritic fused pair-grid MLP on 8 Trainium2 NeuronCores — v2 (fp16).

Math (reference):
    hx = x @ W1[:DX]                      # [B, H]
    hy = y @ W1[DX:] + b1                 # [B, H]
    h  = relu(hx[:,None,:] + hy[None,:,:])        # [B, B, H]
    h2 = relu(h @ W2 + b2)                        # [B, B, H]
    out[i, j] = (h2 @ W3)[i, j, 0] + b3           # [B, B]

Sharding: data-parallel over i (x rows). Each of the 8 cores computes a
[64, 512] slab of scores. y / W1 / W2 / W3 / b* are replicated.

v2 changes vs v1:
  * All matmul operands and SBUF activations are fp16 (PSUM stays f32).
    Halves SBUF read/write bandwidth on every engine and avoids fp32r's
    hardware throughput cliff. Error budget: fp16 rounding ~5e-4 per
    tensor, fp32 accumulation -> ~1e-3 total vs the 2e-2 gate.
  * Input pack is fp16 and loaded by TWO parallel HWDGE DMAs (SP +
    Activation queues) instead of one SWDGE DMA; the small f32 bias pack
    rides the SP queue first. Prep operands land on one queue, W2/W3 on
    the other, so prep GEMMs start before the whole pack arrives.

Per-core dataflow:
    prep:  hyT[h, j]  = (W1y.T @ yT)           [256, 512] fp16 in SBUF
           bias[h, i] = (W1x.T @ xT) + b1      [256, 64]  f32  in SBUF
    per i: A_iT[h, j]  = relu(hyT + bias[:, i])     (DVE, fp16 out)
           Z_iT[m, j]  = W2.T @ A_iT   (PSUM f32)   (4 matmuls, N=512)
           Z2_iT[m, j] = relu(Z_iT + b2)            (ACT, fp16 out)
           score[i, j] += W3.T @ Z2_iT              (2 matmuls)
    The 128 score matvecs accumulate into ONE [64, 512] PSUM tile via the
    shifted one-hot W3 trick (matvec i writes only partition i). b3 is
    added on the host after the gather.

Sync-wait discipline: walrus permits only ONE semaphore wait per compute
instruction. Each engine "pre-touches" every DMA'd region once so later
instructions never need a fresh DMA wait; excess waits are spilled into
EventSemaphore instructions by _legalize_waits.
"""

import numpy as np

B = 512
DX = 128
DY = 128
H = 256
P = 128          # partitions
HC = H // P      # h chunks (2)
NCORES = 8
BS = B // NCORES  # 64 rows of x per core

# fp16 pack column offsets (fp16 words per partition)
# queue A (SP):  prep operands  [OFF_YT .. OFF_W1Y+H)
# queue B (ACT): W2 + W3 trick  [OFF_W2 .. OFF_W3T+2P)
OFF_YT = 0            # [512]     y.T
OFF_XT = 512          # [64]      x_shard.T
OFF_W1X = 576         # [256]     W1[:DX]
OFF_W1Y = 832         # [256]     W1[DX:]
SPLIT_A = 1088        # end of queue-A region
OFF_W2 = 1088         # [2, 256]  W2 k-chunks      (lhsT slices [128,128])
OFF_W3T = 1600        # [2, 128]  one-hot W3 trick
PACK16_COLS = 1856

PACK32_COLS = 4       # b1 (2 cols), b2 (2 cols)

_cache = {}


def _build_nc(legalize=True, reps=1, loop_reps=0):
    import concourse.bass as bass
    import concourse.tile as tile
    import concourse.mybir as mybir

    f32 = mybir.dt.float32
    f16 = mybir.dt.float16
    Alu = mybir.AluOpType
    Act = mybir.ActivationFunctionType

    nc = bass.Bass(
        trn_type="TRN2",
        target_bir_lowering=False,
        debug=False,
        num_devices=NCORES,
    )

    d_pack16 = nc.dram_tensor("pack16", [P, PACK16_COLS], f16, kind="ExternalInput")
    d_pack32 = nc.dram_tensor("pack32", [P, PACK32_COLS], f32, kind="ExternalInput")
    d_out = nc.dram_tensor("out", [BS, B], f16, kind="ExternalOutput")

    with tile.TileContext(nc) as tc:
        with (
            tc.tile_pool(name="singles", bufs=1) as singles,
            tc.tile_pool(name="apool0", bufs=8) as apool0,
            tc.tile_pool(name="apool1", bufs=8) as apool1,
            tc.tile_pool(name="z2pool0", bufs=6) as z2pool0,
            tc.tile_pool(name="z2pool1", bufs=6) as z2pool1,
            tc.tile_pool(name="zpool", bufs=3, space="PSUM") as zpool,
            tc.tile_pool(name="spool", bufs=1, space="PSUM") as spool,
        ):
            pk = singles.tile([P, PACK16_COLS], f16)
            pk32 = singles.tile([P, PACK32_COLS], f32)
            nc.sync.dma_start(pk32[:], d_pack32[:])
            nc.sync.dma_start(pk[:, :SPLIT_A], d_pack16[:, :SPLIT_A])
            nc.scalar.dma_start(pk[:, SPLIT_A:], d_pack16[:, SPLIT_A:])

            def w2_lhsT(c, m):
                return pk[:, OFF_W2 + c * H + m * P: OFF_W2 + c * H + (m + 1) * P]

            def w3_lhsT(c, i):
                o = OFF_W3T + c * P + P // 2 - i
                return pk[:, o: o + BS]

            def b1_col(c):
                return pk32[:, c: c + 1]

            def b2_col(c):
                return pk32[:, HC + c: HC + c + 1]

            # pre-touch: one op per engine per DMA'd region it reads, so
            # later instructions never need a fresh DMA wait.
            scratch = singles.tile([P, 4], f32)
            nc.vector.tensor_copy(scratch[:, 0:1], b1_col(0))
            nc.scalar.copy(scratch[:, 1:2], b2_col(0))
            nc.vector.tensor_copy(scratch[:, 2:3].bitcast(f16), pk[:, 0:2])
            nc.scalar.copy(scratch[:, 3:4].bitcast(f16), pk[:, SPLIT_A:SPLIT_A + 2])

            sb_hy = singles.tile([P, HC, B], f16)
            sb_bias = singles.tile([P, HC, BS], f32)
            outbuf = singles.tile([BS, B], f16)

            # ---- prep: hyT and per-row bias ----
            for c in range(HC):
                ps_hy = zpool.tile([P, B], f32, tag="z")
                nc.tensor.matmul(
                    ps_hy[:],
                    pk[:, OFF_W1Y + c * P: OFF_W1Y + (c + 1) * P],
                    pk[:, OFF_YT: OFF_YT + B],
                    start=True,
                    stop=True,
                )
                nc.vector.tensor_copy(sb_hy[:, c, :], ps_hy[:])

                ps_hx = zpool.tile([P, BS], f32, tag="z")
                nc.tensor.matmul(
                    ps_hx[:],
                    pk[:, OFF_W1X + c * P: OFF_W1X + (c + 1) * P],
                    pk[:, OFF_XT: OFF_XT + BS],
                    start=True,
                    stop=True,
                )
                nc.vector.tensor_scalar(
                    sb_bias[:, c, :], ps_hx[:], b1_col(c), None, Alu.add
                )

            score_ps = spool.tile([BS, B], f32)

            # ---- main loop over the 64 x-rows of this core ----
            # (reps>1 replicates the whole loop for slope-based device
            # timing; each rep restarts the score accumulation;
            # loop_reps>0 wraps the body in a device-side For_i instead)
            import contextlib
            loop_cm = (
                tc.For_i(0, loop_reps, 1) if loop_reps
                else contextlib.nullcontext()
            )
            # Software pipeline: row idx's W3 score matvecs are emitted
            # after row idx+1's W2 matmuls, so the PE never waits on the
            # ACT relu of the row it is scoring (z2 is ~a full W2 stage
            # old by then). `pending` holds (i, z20, z21) of the row whose
            # W3 stage hasn't been emitted yet.
            with loop_cm:
              for rep in range(reps):
                pend_q = []
                n_sc = 0  # score matmuls emitted (for start flag)
                for idx in range(BS):
                  i = (idx % 2) * (BS // 2) + idx // 2
                  a0 = apool0.tile([P, B], f16, tag="a0")
                  a1 = apool1.tile([P, B], f16, tag="a1")
                  a = [a0, a1]
                  nc.vector.tensor_scalar(
                      a1[:], sb_hy[:, 1, :], sb_bias[:, 1, i:i + 1],
                      0.0, Alu.add, Alu.max,
                  )
                  nc.vector.tensor_scalar(
                      a0[:], sb_hy[:, 0, :], sb_bias[:, 0, i:i + 1],
                      0.0, Alu.add, Alu.max,
                  )

                  z = zpool.tile([P, HC, B], f32, tag="z")
                  for m in range(HC):
                      for c in range(HC):
                          nc.tensor.matmul(
                              z[:, m, :],
                              w2_lhsT(c, m),
                              a[c][:],
                              start=(c == 0),
                              stop=(c == HC - 1),
                              skip_group_check=True,
                          )

                  if len(pend_q) >= 2:
                      pi, pz2 = pend_q.pop(0)
                      for c in range(HC):
                          nc.tensor.matmul(
                              score_ps[:],
                              w3_lhsT(c, pi),
                              pz2[c][:],
                              start=(n_sc == 0),
                              stop=False,
                              skip_group_check=True,
                          )
                          n_sc += 1

                  z20 = z2pool0.tile([P, B], f16, tag="z20")
                  z21 = z2pool1.tile([P, B], f16, tag="z21")
                  nc.scalar.activation(
                      z21[:], z[:, 1, :], Act.Relu, bias=b2_col(1), scale=1.0
                  )
                  nc.scalar.activation(
                      z20[:], z[:, 0, :], Act.Relu, bias=b2_col(0), scale=1.0
                  )
                  pend_q.append((i, [z20, z21]))

                # drain: W3 for the remaining rows
                n_drain = len(pend_q)
                for k in range(n_drain):
                    pi, pz2 = pend_q.pop(0)
                    for c in range(HC):
                        nc.tensor.matmul(
                            score_ps[:],
                            w3_lhsT(c, pi),
                            pz2[c][:],
                            start=(n_sc == 0),
                            stop=(k == n_drain - 1 and c == HC - 1),
                            skip_group_check=True,
                        )
                        n_sc += 1

            nc.vector.tensor_copy(outbuf[:], score_ps[:])
            nc.sync.dma_start(d_out[:, :B // 2], outbuf[:, :B // 2])
            nc.scalar.dma_start(d_out[:, B // 2:], outbuf[:, B // 2:])

    if legalize:
        _legalize_waits(nc)
    return nc


def _legalize_waits(nc):
    """walrus accepts only ONE sync wait per compute instruction (and two
    per EventSemaphore). First DROP same-engine self-waits (a wait on the
    engine's own completion semaphore with threshold <= the number of
    preceding instructions on that queue is trivially satisfied by queue
    order); then spill any remaining excess into EventSemaphore
    instructions inserted just before the op on the same engine queue."""
    import concourse.mybir as mybir

    # For each semaphore, track (a) how much it has been incremented by
    # PRECEDING instructions on each engine queue, and (b) whether any
    # other engine also updates it. A wait on a sem that is updated only
    # by the waiting instruction's own queue, with threshold <= the
    # increments already emitted before it, is trivially satisfied by
    # in-order queue execution and can be dropped.
    sem_engines = {}
    for f in nc.m.functions:
        for bb in f.blocks:
            for inst in bb.instructions:
                si = inst.sync_info
                if si is None:
                    continue
                for u in si.on_update or []:
                    sem_engines.setdefault(u.ant_name, set()).add(str(inst.engine))

    n_spilled = 0
    for f in nc.m.functions:
        for bb in f.blocks:
            upd = {}
            insts = bb.instructions
            i = 0
            while i < len(insts):
                inst = insts[i]
                eng = str(inst.engine)
                si = inst.sync_info
                if si is None:
                    i += 1
                    continue
                waits = list(si.on_wait or [])
                if len(waits) > 1:
                    kept = [
                        w for w in waits
                        if not (
                            w.wait_mode == "sem-ge-imm"
                            and sem_engines.get(w.ant_name) == {eng}
                            and (w.wait_value or 0) <= upd.get(w.ant_name, 0)
                        )
                    ]
                    if kept:
                        waits = kept
                if len(waits) != len(si.on_wait or []):
                    inst.sync_info = mybir.SyncInfo(
                        on_wait=waits, on_update=list(si.on_update or [])
                    )
                    si = inst.sync_info
                for u in si.on_update or []:
                    if u.update_mode == "sem-inc":
                        upd[u.ant_name] = upd.get(u.ant_name, 0) + (
                            u.update_value or 0
                        )
                if not si.on_wait or len(si.on_wait) <= 1 or (
                    inst.opcode == "EventSemaphore"
                ):
                    i += 1
                    continue
                keep, spill = waits[-1], waits[:-1]
                k = 0
                while spill:
                    chunk, spill = spill[:2], spill[2:]
                    ev = mybir.InstEventSemaphore(
                        name=f"{inst.name}-lw{k}", ins=[], outs=[]
                    )
                    ev.engine = inst.engine
                    ev.sync_info = mybir.SyncInfo(on_wait=chunk, on_update=[])
                    insts.insert(i, ev)
                    i += 1
                    k += 1
                    n_spilled += 1
                inst.sync_info = mybir.SyncInfo(
                    on_wait=[keep], on_update=list(si.on_update or [])
                )
                i += 1
    return n_spilled


def prep_inputs(x, y, W1, b1, W2, b2, W3):
    """Host-side sharding/layout. Returns per-core input maps."""
    x = np.ascontiguousarray(np.asarray(x, dtype=np.float32))
    y = np.ascontiguousarray(np.asarray(y, dtype=np.float32))
    W1 = np.asarray(W1, dtype=np.float32)
    b1 = np.asarray(b1, dtype=np.float32)
    W2 = np.asarray(W2, dtype=np.float32)
    b2 = np.asarray(b2, dtype=np.float32)
    W3 = np.asarray(W3, dtype=np.float32)

    pack = np.zeros((P, PACK16_COLS), dtype=np.float16)
    pack[:, OFF_YT:OFF_YT + B] = y.T
    pack[:, OFF_W1X:OFF_W1X + H] = W1[:DX]
    pack[:, OFF_W1Y:OFF_W1Y + H] = W1[DX:]
    # W2 k-chunks: pack[p, OFF_W2 + c*H + j] = W2[c*P + p, j]
    pack[:, OFF_W2:OFF_W2 + HC * H] = (
        np.transpose(W2.reshape(HC, P, H), (1, 0, 2)).reshape(P, HC * H)
    )
    # one-hot W3 trick: column 64 of each [128,128] region holds W3 chunk c
    for c in range(HC):
        pack[:, OFF_W3T + c * P + P // 2] = W3[c * P:(c + 1) * P, 0]

    pack32 = np.zeros((P, PACK32_COLS), dtype=np.float32)
    pack32[:, 0:HC] = b1.reshape(HC, P).T
    pack32[:, HC:2 * HC] = b2.reshape(HC, P).T

    in_maps = []
    for core in range(NCORES):
        pc = pack.copy()
        pc[:, OFF_XT:OFF_XT + BS] = x[core * BS:(core + 1) * BS].T.astype(np.float16)
        in_maps.append({"pack16": pc, "pack32": pack32})
    return in_maps


def kernel(x, y, W1, b1, W2, b2, W3, b3):
    from concourse.bass_utils import run_bass_kernel_spmd

    if "nc" not in _cache:
        _cache["nc"] = _build_nc()
    nc = _cache["nc"]

    in_maps = prep_inputs(x, y, W1, b1, W2, b2, W3)
    res = run_bass_kernel_spmd(nc, in_maps, core_ids=list(range(NCORES)))
    out = np.concatenate(
        [res.results[c]["out"].astype(np.float32) for c in range(NCORES)], axis=0
    )
    out = out + np.float32(np.asarray(b3, dtype=np.float32).reshape(()))
    return out.astype(np.float32)
